# revision 10
# baseline (speedup 1.0000x reference)
"""Trainium2 Bass kernel for the LSTM decoder problem (nn_Decoder).

Math (reference):
    h0 = latent @ W_fc.T + b_fc ;  c0 = 0 ;  x0 = obs_s[-1]
    for t in 0..13:
        gates = x @ W_ih.T + h @ W_hh.T + (b_ih + b_hh)      # [B, 4H], order i,f,g,o
        c = sig(f)*c + sig(i)*tanh(g)
        h = sig(o)*tanh(c)
        x = h @ W_mlp.T + b_mlp                              # [B, 39] -> output step t

Algebraic folds:
  * t>=1: x_t = W_mlp h_{t-1} + b_mlp, so
        gates_t = W_combo h_{t-1} + b_combo,
        W_combo = W_ih W_mlp + W_hh,  b_combo = b_ih + b_hh + W_ih b_mlp.
  * t=0: h0 only feeds gates_0, so
        gates_0 = (W_hh W_fc) latent + xt,
        xt = x0 W_ih.T + b_ih + b_hh + W_hh b_fc   (precomputed on host).
    The h0 phase never runs on device; the recurrence starts from (c1, h1).

Device layout: batch data-parallel over 8 cores (16384 each); per core
NSC=4 superchunks of GROUPS=4 batch groups x C=1024 columns. Activations
live as [128 partitions = 4 groups x 32 dims, C cols]. Gate matmuls use
block-diagonal f16 stationary weights so one matmul serves all 4 groups.

Engine budget per (t, sc): ACT does the 5 transcendentals (the wall: 4
gate activations reading PSUM + tanh(c) reading SBUF, all [128, C]); DVE
does the f16 cell arithmetic (2x packed mode) + half the mlp-PSUM
evictions; Pool does the other evictions. The eviction is a tensor_scalar
add of b_mlp that converts f32 PSUM -> f16 SBUF. 4 superchunks interleave
so the per-chunk recurrence latency (tanh_c -> h -> matmul -> act) hides
behind the other chunks' ACT work. Output DMA is f16 [T, NSC, 2, 78, C],
upcast + bias on host in assemble_output.
"""

import numpy as np
from contextlib import ExitStack

import concourse.bass as bass
import concourse.bacc as bacc
import concourse.tile as tile
from concourse import mybir
from concourse.bass_utils import run_bass_kernel_spmd

POSE, H, LATD = 39, 32, 16
B_TOTAL, T = 131072, 14
NCORES = 8
BS = B_TOTAL // NCORES          # 16384 batch per core
NSC = 4                         # superchunks per core
GROUPS = 4                      # batch groups stacked on partitions
C = BS // (NSC * GROUPS)        # 1024 columns per group per superchunk
MMW = 512                       # matmul moving free dim (one PSUM bank)
# packed-constant column offsets (fp16 weight pack):
#   wg   4 x [128,128]  block-diag W_combo.T per gate
#   whf  3 x [64,128]   block-diag (W_hh[g] W_fc).T per t0 gate (i, g~, o)
#   wmlp     [128,78]   block-diag W_mlp.T per group-pair
OW_G, OW_HF, OW_MLP, OW_ID = 0, 512, 896, 974
WPACK_COLS = 1102
T0_GATES = (0, 2, 3)            # f-gate skipped at t=0 (c0 = 0)

F32 = mybir.dt.float32
F16 = mybir.dt.float16
SIG = mybir.ActivationFunctionType.Sigmoid
TANH = mybir.ActivationFunctionType.Tanh
MULT = mybir.AluOpType.mult
ADD = mybir.AluOpType.add


def _build_body(ctx, tc, io, _step_schedule=tuple(range(T))):
    nc = tc.nc

    consts = ctx.enter_context(tc.tile_pool(name="consts", bufs=1))
    xin = ctx.enter_context(tc.tile_pool(name="xin", bufs=2))
    state = ctx.enter_context(tc.tile_pool(name="state", bufs=1))
    acts = ctx.enter_context(tc.tile_pool(name="acts", bufs=3))
    tmps = ctx.enter_context(tc.tile_pool(name="tmps", bufs=3))
    stg = ctx.enter_context(tc.tile_pool(name="stg", bufs=3))
    psg = ctx.enter_context(tc.tile_pool(name="psg", bufs=3, space="PSUM"))
    psm = ctx.enter_context(tc.tile_pool(name="psm", bufs=2, space="PSUM"))

    # ---- constants to SBUF (packed: 2 DMAs) ----
    wpack_sb = consts.tile([128, WPACK_COLS], F16, tag="wpack", name="wpack")
    bpack_sb = consts.tile([128, 6], F32, tag="bpack", name="bpack")
    nc.sync.dma_start(out=wpack_sb, in_=io["wpack"])
    nc.sync.dma_start(out=bpack_sb, in_=io["bpack"])
    wg_sb = [wpack_sb[:, OW_G + 128 * g : OW_G + 128 * (g + 1)] for g in range(4)]
    whf_sb = [wpack_sb[0:64, OW_HF + 128 * k : OW_HF + 128 * (k + 1)] for k in range(3)]
    wmlp_sb = wpack_sb[:, OW_MLP : OW_MLP + 78]
    ident_sb = wpack_sb[:, OW_ID : OW_ID + 128]
    bgc_sb = bpack_sb[:, 0:4]
    bmlp_sb = bpack_sb[0:78, 4:5]

    # ---- per-superchunk persistent state (f16) ----
    h = [state.tile([128, C], F16, tag=f"h{sc}", name=f"h{sc}") for sc in range(NSC)]
    cst = [state.tile([128, C], F16, tag=f"c{sc}", name=f"c{sc}") for sc in range(NSC)]

    # ---- decode steps ----
    for t in _step_schedule:
        for sc in range(NSC):
            if t == 0:
                lat_sb = xin.tile([64, C], F16, tag="lat", name="lat")
                nc.sync.dma_start(out=lat_sb, in_=io["lat"][sc])
                xt_sb = xin.tile([128, 3, C], F16, tag="xt", name="xt")
                for k in range(3):  # per-gate chunks so gate i can start early
                    nc.sync.dma_start(out=xt_sb[:, k], in_=io["xt"][sc][:, k])
            sig = {}
            for gi, g in enumerate(T0_GATES if t == 0 else range(4)):
                a = acts.tile([128, C], F16, tag=f"a{g}", name=f"a{g}")
                ps = psg.tile([128, C], F32, tag="psg", name="psg")
                for m in range(C // MMW):
                    rcols = slice(m * MMW, (m + 1) * MMW)
                    if t == 0:
                        # gates_0 = (W_hh W_fc) lat + xt; xt (which carries
                        # W_ih x0 + b_ih + b_hh + W_hh b_fc) accumulates into
                        # PSUM through an identity-stationary matmul so no
                        # vector-engine add rides the critical path.
                        nc.tensor.matmul(
                            ps[:, rcols],
                            lhsT=whf_sb[gi],
                            rhs=lat_sb[:, rcols],
                            start=True,
                            stop=False,
                        )
                        nc.tensor.matmul(
                            ps[:, rcols],
                            lhsT=ident_sb,
                            rhs=xt_sb[:, gi, rcols],
                            start=False,
                            stop=True,
                        )
                    else:
                        nc.tensor.matmul(
                            ps[:, rcols],
                            lhsT=wg_sb[g],
                            rhs=h[sc][:, rcols],
                            start=True,
                            stop=True,
                        )
                if t == 0:
                    nc.scalar.activation(a, ps, TANH if g == 2 else SIG)
                else:
                    nc.scalar.activation(
                        a, ps, TANH if g == 2 else SIG, bias=bgc_sb[:, g : g + 1]
                    )
                sig[g] = a
            # LSTM cell update (f16 on DVE; 2x packed mode)
            if t == 0:
                # c0 = 0 -> c1 = sig(i) * tanh(g)
                nc.vector.tensor_tensor(cst[sc], sig[0], sig[2], MULT)
            else:
                t1 = tmps.tile([128, C], F16, tag="t1", name="t1")
                t2 = tmps.tile([128, C], F16, tag="t2", name="t2")
                nc.vector.tensor_tensor(t2, sig[0], sig[2], MULT)
                nc.vector.tensor_tensor(t1, sig[1], cst[sc], MULT)
                nc.vector.tensor_tensor(cst[sc], t1, t2, ADD)
            tct = tmps.tile([128, C], F16, tag="tc", name="tc")
            nc.scalar.activation(tct, cst[sc], TANH)
            nc.vector.tensor_tensor(h[sc], sig[3], tct, MULT)
            # mlp output for this superchunk, per group-pair
            for pr in range(2):
                stage = stg.tile([78, C], F16, tag=f"st{pr}", name=f"st{pr}")
                for m in range(C // MMW):
                    pm = psm.tile([78, MMW], F32, tag="psm", name="psm")
                    rcols = slice(m * MMW, (m + 1) * MMW)
                    nc.tensor.matmul(
                        pm,
                        lhsT=wmlp_sb[64 * pr : 64 * (pr + 1), :],
                        rhs=h[sc][64 * pr : 64 * (pr + 1), rcols],
                        start=True,
                        stop=True,
                    )
                    # evict PSUM -> f16 SBUF (GpSimd cannot read PSUM, so
                    # these all ride DVE); b_mlp is added on the host
                    nc.vector.tensor_copy(stage[:, rcols], pm)
                nc.sync.dma_start(out=io["out"][t, sc, pr], in_=stage)


_NC_CACHE = {}


def build_nc(mode="real"):
    """mode: "real" (grading path), ("timing", reps) (big output -> internal
    DRAM scratch + tiny external output, same HW work), "nop" (RPC floor)."""
    global _NC_CACHE
    if mode in _NC_CACHE:
        return _NC_CACHE[mode]
    nc = bacc.Bacc("TRN2", target_bir_lowering=False, debug=False)
    if mode == "nop":
        tin = nc.dram_tensor("lat", [1, 4], F32, kind="ExternalInput").ap()
        tout = nc.dram_tensor("tout", [1, 4], F32, kind="ExternalOutput").ap()
        with tile.TileContext(nc) as tc:
            with ExitStack() as ctx:
                pool = ctx.enter_context(tc.tile_pool(name="p", bufs=1))
                t = pool.tile([1, 4], F32, tag="t", name="t")
                nc.sync.dma_start(out=t, in_=tin)
                nc.sync.dma_start(out=tout, in_=t)
        nc.compile()
        _NC_CACHE[mode] = nc
        return nc
    io = {
        "lat": nc.dram_tensor("lat", [NSC, 64, C], F16, kind="ExternalInput").ap(),
        "xt": nc.dram_tensor("xt", [NSC, 128, 3, C], F16, kind="ExternalInput").ap(),
        "wpack": nc.dram_tensor("wpack", [128, WPACK_COLS], F16, kind="ExternalInput").ap(),
        "bpack": nc.dram_tensor("bpack", [128, 6], F32, kind="ExternalInput").ap(),
        "out": nc.dram_tensor(
            "out",
            [T, NSC, 2, 78, C],
            F16,
            kind="ExternalOutput" if mode == "real" else "Internal",  # noqa
        ).ap(),
    }
    reps = 1
    if isinstance(mode, tuple):
        reps = mode[1]
    if mode != "real":
        io["tout"] = nc.dram_tensor("tout", [1, 4], F32, kind="ExternalOutput").ap()
    sched = tuple(t for r in range(reps) for t in range(T))
    with tile.TileContext(nc) as tc:
        with ExitStack() as ctx:
            _build_body(ctx, tc, io, sched)
            if mode != "real":
                tpool = ctx.enter_context(tc.tile_pool(name="tp", bufs=1))
                tt = tpool.tile([1, 4], F32, tag="tt", name="tt")
                nc.vector.memset(tt, 1.0)
                nc.sync.dma_start(out=io["tout"], in_=tt)
    nc.compile()
    _NC_CACHE[mode] = nc
    return nc


def prep_inputs(obs_s, latent, W_ih, W_hh, b_ih, b_hh, W_fc, b_fc, W_mlp, b_mlp):
    """Host-side weight folding + sharding. Returns per-core input maps."""
    f32, f16 = np.float32, np.float16
    W_ih = np.asarray(W_ih, f32)
    W_hh = np.asarray(W_hh, f32)
    b_ih = np.asarray(b_ih, f32)
    b_hh = np.asarray(b_hh, f32)
    W_fc = np.asarray(W_fc, f32)
    b_fc = np.asarray(b_fc, f32)
    W_mlp = np.asarray(W_mlp, f32)
    b_mlp = np.asarray(b_mlp, f32)

    W_combo = W_ih @ W_mlp + W_hh                    # [4H, H]
    b_combo = b_ih + b_hh + W_ih @ b_mlp             # [4H]

    wg = np.zeros((4, 128, 128), f32)
    for g in range(4):
        for j in range(4):
            wg[g, 32 * j : 32 * (j + 1), 32 * j : 32 * (j + 1)] = W_combo[
                32 * g : 32 * (g + 1)
            ].T
    # t0: gates_0 = (W_hh W_fc) latent + xt ; block-diag over 4 groups of
    # 16 latent dims -> 32 gate dims, for gates i, g~, o only
    whf = np.zeros((3, 64, 128), f32)
    for k, g in enumerate(T0_GATES):
        blk = (W_hh[32 * g : 32 * (g + 1)] @ W_fc).T      # [16, 32]
        for j in range(4):
            whf[k, 16 * j : 16 * (j + 1), 32 * j : 32 * (j + 1)] = blk
    wmlp = np.zeros((128, 78), f32)
    for half in range(2):
        for j in range(2):
            wmlp[
                64 * half + 32 * j : 64 * half + 32 * (j + 1),
                39 * j : 39 * (j + 1),
            ] = W_mlp.T
    bgc = np.stack([np.tile(b_combo[32 * g : 32 * (g + 1)], 4) for g in range(4)])
    bmlp_v = np.tile(b_mlp, 2).astype(f32)

    # xt = x0 W_ih.T + b_ih + b_hh + W_hh b_fc  (t0 gate constant), [B, 4H]
    x0 = np.asarray(obs_s[-1], f32)                       # [B, 39]
    xt_full = x0 @ W_ih.T + (b_ih + b_hh + W_hh @ b_fc)   # [B, 128]
    latT = np.ascontiguousarray(np.asarray(latent, f32).T).astype(f16)  # [16, B]

    wpack = np.zeros((128, WPACK_COLS), f32)
    for g in range(4):
        wpack[:, OW_G + 128 * g : OW_G + 128 * (g + 1)] = wg[g]
    for k in range(3):
        wpack[:64, OW_HF + 128 * k : OW_HF + 128 * (k + 1)] = whf[k]
    wpack[:, OW_MLP : OW_MLP + 78] = wmlp
    wpack[:, OW_ID : OW_ID + 128] = np.eye(128, dtype=f32)
    bpack = np.zeros((128, 6), f32)
    bpack[:, 0:4] = bgc.T
    bpack[:78, 4] = bmlp_v
    common = {"wpack": wpack.astype(f16), "bpack": bpack}
    in_maps = []
    for c in range(NCORES):
        base = c * BS
        lp = np.empty((NSC, 64, C), f16)
        xp = np.empty((NSC, 128, 3, C), f16)
        for sc in range(NSC):
            for j in range(GROUPS):
                s = base + sc * GROUPS * C + j * C
                lp[sc, 16 * j : 16 * (j + 1), :] = latT[:, s : s + C]
                for k, g in enumerate(T0_GATES):
                    xp[sc, 32 * j : 32 * (j + 1), k, :] = xt_full[
                        s : s + C, 32 * g : 32 * (g + 1)
                    ].T
        m = dict(common)
        m["lat"] = lp
        m["xt"] = xp
        in_maps.append(m)
    return in_maps


def assemble_output(per_core_out, b_mlp):
    """per_core_out: list of [T, NSC, 2, 78, C] f16 arrays -> [T, B, 39] f32.

    Device output omits the b_mlp bias (folded out of the recurrence);
    add it here during the upcast/transpose pass.
    """
    b_mlp = np.asarray(b_mlp, np.float32)
    preds = np.empty((T, B_TOTAL, POSE), np.float32)
    for c in range(NCORES):
        arr = np.asarray(per_core_out[c], np.float32)
        a = (
            arr.reshape(T, NSC, 2, 2, POSE, C)
            .transpose(0, 1, 2, 3, 5, 4)
            .reshape(T, BS, POSE)
        )
        preds[:, c * BS : (c + 1) * BS] = a + b_mlp
    return preds


def kernel(obs_s, latent, W_ih, W_hh, b_ih, b_hh, W_fc, b_fc, W_mlp, b_mlp, pred_len):
    assert int(pred_len) == T, f"kernel hardcodes pred_len={T}, got {pred_len}"
    in_maps = prep_inputs(
        obs_s, latent, W_ih, W_hh, b_ih, b_hh, W_fc, b_fc, W_mlp, b_mlp
    )
    nc = build_nc()
    res = run_bass_kernel_spmd(nc, in_maps, core_ids=list(range(NCORES)))
    return assemble_output([res.results[c]["out"] for c in range(NCORES)], b_mlp)


# revision 11
# speedup vs baseline: 1.1037x; 1.1037x over previous
"""Trainium2 Bass kernel for the LSTM decoder problem (nn_Decoder).

Math (reference):
    h0 = latent @ W_fc.T + b_fc ;  c0 = 0 ;  x0 = obs_s[-1]
    for t in 0..13:
        gates = x @ W_ih.T + h @ W_hh.T + (b_ih + b_hh)      # [B, 4H], order i,f,g,o
        c = sig(f)*c + sig(i)*tanh(g)
        h = sig(o)*tanh(c)
        x = h @ W_mlp.T + b_mlp                              # [B, 39] -> output step t

Algebraic folds:
  * t>=1: x_t = W_mlp h_{t-1} + b_mlp, so
        gates_t = W_combo h_{t-1} + b_combo,
        W_combo = W_ih W_mlp + W_hh,  b_combo = b_ih + b_hh + W_ih b_mlp.
  * t=0: h0 only feeds gates_0, so
        gates_0 = (W_hh W_fc) latent + xt,
        xt = x0 W_ih.T + b_ih + b_hh + W_hh b_fc   (precomputed on host).
    The h0 phase never runs on device; the recurrence starts from (c1, h1).

Device layout: batch data-parallel over 8 cores (16384 each); per core
NSC=4 superchunks of GROUPS=4 batch groups x C=1024 columns. Activations
live as [128 partitions = 4 groups x 32 dims, C cols]. Gate matmuls use
block-diagonal f16 stationary weights so one matmul serves all 4 groups.

Engine budget per (t, sc): ACT does the 5 transcendentals (the wall: 4
gate activations reading PSUM + tanh(c) reading SBUF, all [128, C]); DVE
does the f16 cell arithmetic (2x packed mode) + half the mlp-PSUM
evictions; Pool does the other evictions. The eviction is a tensor_scalar
add of b_mlp that converts f32 PSUM -> f16 SBUF. 4 superchunks interleave
so the per-chunk recurrence latency (tanh_c -> h -> matmul -> act) hides
behind the other chunks' ACT work. Output DMA is f16 [T, NSC, 2, 78, C],
upcast + bias on host in assemble_output.
"""

import numpy as np
from contextlib import ExitStack

import concourse.bass as bass
import concourse.bacc as bacc
import concourse.tile as tile
from concourse import mybir
from concourse.bass_utils import run_bass_kernel_spmd

POSE, H, LATD = 39, 32, 16
B_TOTAL, T = 131072, 14
NCORES = 8
BS = B_TOTAL // NCORES          # 16384 batch per core
NSC = 4                         # superchunks per core
GROUPS = 4                      # batch groups stacked on partitions
C = BS // (NSC * GROUPS)        # 1024 columns per group per superchunk
MMW = 512                       # matmul moving free dim (one PSUM bank)
# packed-constant column offsets (fp16 weight pack):
#   wg   4 x [128,128]  block-diag W_combo.T per gate
#   whf  3 x [64,128]   block-diag (W_hh[g] W_fc).T per t0 gate (i, g~, o)
#   wmlp     [128,78]   block-diag W_mlp.T per group-pair
OW_G, OW_HF, OW_MLP, OW_ID = 0, 512, 896, 974
WPACK_COLS = 1102
T0_GATES = (0, 2, 3)            # f-gate skipped at t=0 (c0 = 0)

F32 = mybir.dt.float32
F16 = mybir.dt.float16
SIG = mybir.ActivationFunctionType.Sigmoid
TANH = mybir.ActivationFunctionType.Tanh
MULT = mybir.AluOpType.mult
ADD = mybir.AluOpType.add


def _build_body(ctx, tc, io, _step_schedule=tuple(range(T))):
    nc = tc.nc

    consts = ctx.enter_context(tc.tile_pool(name="consts", bufs=1))
    xin = ctx.enter_context(tc.tile_pool(name="xin", bufs=2))
    state = ctx.enter_context(tc.tile_pool(name="state", bufs=1))
    acts = ctx.enter_context(tc.tile_pool(name="acts", bufs=3))
    tmps = ctx.enter_context(tc.tile_pool(name="tmps", bufs=3))
    stg = ctx.enter_context(tc.tile_pool(name="stg", bufs=3))
    psg = ctx.enter_context(tc.tile_pool(name="psg", bufs=3, space="PSUM"))
    psm = ctx.enter_context(tc.tile_pool(name="psm", bufs=2, space="PSUM"))

    # ---- constants to SBUF (packed: 2 DMAs) ----
    wpack_sb = consts.tile([128, WPACK_COLS], F16, tag="wpack", name="wpack")
    bpack_sb = consts.tile([128, 6], F32, tag="bpack", name="bpack")
    nc.sync.dma_start(out=wpack_sb, in_=io["wpack"])
    nc.sync.dma_start(out=bpack_sb, in_=io["bpack"])
    wg_sb = [wpack_sb[:, OW_G + 128 * g : OW_G + 128 * (g + 1)] for g in range(4)]
    whf_sb = [wpack_sb[0:64, OW_HF + 128 * k : OW_HF + 128 * (k + 1)] for k in range(3)]
    wmlp_sb = wpack_sb[:, OW_MLP : OW_MLP + 78]
    ident_sb = wpack_sb[:, OW_ID : OW_ID + 128]
    bgc_sb = bpack_sb[:, 0:4]
    bmlp_sb = bpack_sb[0:78, 4:5]

    # ---- per-superchunk persistent state (f16) ----
    h = [state.tile([128, C], F16, tag=f"h{sc}", name=f"h{sc}") for sc in range(NSC)]
    cst = [state.tile([128, C], F16, tag=f"c{sc}", name=f"c{sc}") for sc in range(NSC)]

    # ---- decode steps ----
    pending_mlp = []

    def _emit_mlp(key):
        t, sc = key
        for pr in range(2):
            stage = stg.tile([78, C], F16, tag=f"st{pr}", name=f"st{pr}")
            for m in range(C // MMW):
                pm = psm.tile([78, MMW], F32, tag="psm", name="psm")
                rcols = slice(m * MMW, (m + 1) * MMW)
                nc.tensor.matmul(
                    pm,
                    lhsT=wmlp_sb[64 * pr : 64 * (pr + 1), :],
                    rhs=h[sc][64 * pr : 64 * (pr + 1), rcols],
                    start=True,
                    stop=True,
                )
                # evict PSUM -> f16 SBUF (GpSimd cannot read PSUM, so these
                # all ride DVE); b_mlp is added on the host
                nc.vector.tensor_copy(stage[:, rcols], pm)
            nc.sync.dma_start(out=io["out"][t, sc, pr], in_=stage)

    for t in _step_schedule:
        for sc in range(NSC):
            if t == 0:
                lat_sb = xin.tile([64, C], F16, tag="lat", name="lat")
                nc.sync.dma_start(out=lat_sb, in_=io["lat"][sc])
                xt_sb = xin.tile([128, 3, C], F16, tag="xt", name="xt")
                for k in range(3):  # per-gate chunks so gate i can start early
                    nc.sync.dma_start(out=xt_sb[:, k], in_=io["xt"][sc][:, k])
            sig = {}
            for gi, g in enumerate(T0_GATES if t == 0 else range(4)):
                a = acts.tile([128, C], F16, tag=f"a{g}", name=f"a{g}")
                ps = psg.tile([128, C], F32, tag="psg", name="psg")
                for m in range(C // MMW):
                    rcols = slice(m * MMW, (m + 1) * MMW)
                    if t == 0:
                        # gates_0 = (W_hh W_fc) lat + xt; xt (which carries
                        # W_ih x0 + b_ih + b_hh + W_hh b_fc) accumulates into
                        # PSUM through an identity-stationary matmul so no
                        # vector-engine add rides the critical path.
                        nc.tensor.matmul(
                            ps[:, rcols],
                            lhsT=whf_sb[gi],
                            rhs=lat_sb[:, rcols],
                            start=True,
                            stop=False,
                        )
                        nc.tensor.matmul(
                            ps[:, rcols],
                            lhsT=ident_sb,
                            rhs=xt_sb[:, gi, rcols],
                            start=False,
                            stop=True,
                        )
                    else:
                        nc.tensor.matmul(
                            ps[:, rcols],
                            lhsT=wg_sb[g],
                            rhs=h[sc][:, rcols],
                            start=True,
                            stop=True,
                        )
                if t == 0:
                    nc.scalar.activation(a, ps, TANH if g == 2 else SIG)
                else:
                    nc.scalar.activation(
                        a, ps, TANH if g == 2 else SIG, bias=bgc_sb[:, g : g + 1]
                    )
                sig[g] = a
            # LSTM cell update (f16 on DVE; 2x packed mode)
            if t == 0:
                # c0 = 0 -> c1 = sig(i) * tanh(g)
                nc.vector.tensor_tensor(cst[sc], sig[0], sig[2], MULT)
            else:
                t1 = tmps.tile([128, C], F16, tag="t1", name="t1")
                t2 = tmps.tile([128, C], F16, tag="t2", name="t2")
                nc.vector.tensor_tensor(t2, sig[0], sig[2], MULT)
                nc.vector.tensor_tensor(t1, sig[1], cst[sc], MULT)
                nc.vector.tensor_tensor(cst[sc], t1, t2, ADD)
            tct = tmps.tile([128, C], F16, tag="tc", name="tc")
            nc.scalar.activation(tct, cst[sc], TANH)
            nc.vector.tensor_tensor(h[sc], sig[3], tct, MULT)
            pending_mlp.append((t, sc))
            # Emit the PREVIOUS superchunk's mlp now: its h is ready, so in
            # PE program order the matmuls never sit at a not-yet-computed-h
            # fence (which would reset the tensor engine's p-state ramp).
            while len(pending_mlp) > 1:
                _emit_mlp(pending_mlp.pop(0))
    while pending_mlp:
        _emit_mlp(pending_mlp.pop(0))


_NC_CACHE = {}


def build_nc(mode="real"):
    """mode: "real" (grading path), ("timing", reps) (big output -> internal
    DRAM scratch + tiny external output, same HW work), "nop" (RPC floor)."""
    global _NC_CACHE
    if mode in _NC_CACHE:
        return _NC_CACHE[mode]
    nc = bacc.Bacc("TRN2", target_bir_lowering=False, debug=False)
    if mode == "nop":
        tin = nc.dram_tensor("lat", [1, 4], F32, kind="ExternalInput").ap()
        tout = nc.dram_tensor("tout", [1, 4], F32, kind="ExternalOutput").ap()
        with tile.TileContext(nc) as tc:
            with ExitStack() as ctx:
                pool = ctx.enter_context(tc.tile_pool(name="p", bufs=1))
                t = pool.tile([1, 4], F32, tag="t", name="t")
                nc.sync.dma_start(out=t, in_=tin)
                nc.sync.dma_start(out=tout, in_=t)
        nc.compile()
        _NC_CACHE[mode] = nc
        return nc
    io = {
        "lat": nc.dram_tensor("lat", [NSC, 64, C], F16, kind="ExternalInput").ap(),
        "xt": nc.dram_tensor("xt", [NSC, 128, 3, C], F16, kind="ExternalInput").ap(),
        "wpack": nc.dram_tensor("wpack", [128, WPACK_COLS], F16, kind="ExternalInput").ap(),
        "bpack": nc.dram_tensor("bpack", [128, 6], F32, kind="ExternalInput").ap(),
        "out": nc.dram_tensor(
            "out",
            [T, NSC, 2, 78, C],
            F16,
            kind="ExternalOutput" if mode == "real" else "Internal",  # noqa
        ).ap(),
    }
    reps = 1
    if isinstance(mode, tuple):
        reps = mode[1]
    if mode != "real":
        io["tout"] = nc.dram_tensor("tout", [1, 4], F32, kind="ExternalOutput").ap()
    sched = tuple(t for r in range(reps) for t in range(T))
    with tile.TileContext(nc) as tc:
        with ExitStack() as ctx:
            _build_body(ctx, tc, io, sched)
            if mode != "real":
                tpool = ctx.enter_context(tc.tile_pool(name="tp", bufs=1))
                tt = tpool.tile([1, 4], F32, tag="tt", name="tt")
                nc.vector.memset(tt, 1.0)
                nc.sync.dma_start(out=io["tout"], in_=tt)
    nc.compile()
    _NC_CACHE[mode] = nc
    return nc


def prep_inputs(obs_s, latent, W_ih, W_hh, b_ih, b_hh, W_fc, b_fc, W_mlp, b_mlp):
    """Host-side weight folding + sharding. Returns per-core input maps."""
    f32, f16 = np.float32, np.float16
    W_ih = np.asarray(W_ih, f32)
    W_hh = np.asarray(W_hh, f32)
    b_ih = np.asarray(b_ih, f32)
    b_hh = np.asarray(b_hh, f32)
    W_fc = np.asarray(W_fc, f32)
    b_fc = np.asarray(b_fc, f32)
    W_mlp = np.asarray(W_mlp, f32)
    b_mlp = np.asarray(b_mlp, f32)

    W_combo = W_ih @ W_mlp + W_hh                    # [4H, H]
    b_combo = b_ih + b_hh + W_ih @ b_mlp             # [4H]

    wg = np.zeros((4, 128, 128), f32)
    for g in range(4):
        for j in range(4):
            wg[g, 32 * j : 32 * (j + 1), 32 * j : 32 * (j + 1)] = W_combo[
                32 * g : 32 * (g + 1)
            ].T
    # t0: gates_0 = (W_hh W_fc) latent + xt ; block-diag over 4 groups of
    # 16 latent dims -> 32 gate dims, for gates i, g~, o only
    whf = np.zeros((3, 64, 128), f32)
    for k, g in enumerate(T0_GATES):
        blk = (W_hh[32 * g : 32 * (g + 1)] @ W_fc).T      # [16, 32]
        for j in range(4):
            whf[k, 16 * j : 16 * (j + 1), 32 * j : 32 * (j + 1)] = blk
    wmlp = np.zeros((128, 78), f32)
    for half in range(2):
        for j in range(2):
            wmlp[
                64 * half + 32 * j : 64 * half + 32 * (j + 1),
                39 * j : 39 * (j + 1),
            ] = W_mlp.T
    bgc = np.stack([np.tile(b_combo[32 * g : 32 * (g + 1)], 4) for g in range(4)])
    bmlp_v = np.tile(b_mlp, 2).astype(f32)

    # xt = x0 W_ih.T + b_ih + b_hh + W_hh b_fc  (t0 gate constant), [B, 4H]
    x0 = np.asarray(obs_s[-1], f32)                       # [B, 39]
    xt_full = x0 @ W_ih.T + (b_ih + b_hh + W_hh @ b_fc)   # [B, 128]
    latT = np.ascontiguousarray(np.asarray(latent, f32).T).astype(f16)  # [16, B]

    wpack = np.zeros((128, WPACK_COLS), f32)
    for g in range(4):
        wpack[:, OW_G + 128 * g : OW_G + 128 * (g + 1)] = wg[g]
    for k in range(3):
        wpack[:64, OW_HF + 128 * k : OW_HF + 128 * (k + 1)] = whf[k]
    wpack[:, OW_MLP : OW_MLP + 78] = wmlp
    wpack[:, OW_ID : OW_ID + 128] = np.eye(128, dtype=f32)
    bpack = np.zeros((128, 6), f32)
    bpack[:, 0:4] = bgc.T
    bpack[:78, 4] = bmlp_v
    common = {"wpack": wpack.astype(f16), "bpack": bpack}
    in_maps = []
    for c in range(NCORES):
        base = c * BS
        lp = np.empty((NSC, 64, C), f16)
        xp = np.empty((NSC, 128, 3, C), f16)
        for sc in range(NSC):
            for j in range(GROUPS):
                s = base + sc * GROUPS * C + j * C
                lp[sc, 16 * j : 16 * (j + 1), :] = latT[:, s : s + C]
                for k, g in enumerate(T0_GATES):
                    xp[sc, 32 * j : 32 * (j + 1), k, :] = xt_full[
                        s : s + C, 32 * g : 32 * (g + 1)
                    ].T
        m = dict(common)
        m["lat"] = lp
        m["xt"] = xp
        in_maps.append(m)
    return in_maps


def assemble_output(per_core_out, b_mlp):
    """per_core_out: list of [T, NSC, 2, 78, C] f16 arrays -> [T, B, 39] f32.

    Device output omits the b_mlp bias (folded out of the recurrence);
    add it here during the upcast/transpose pass.
    """
    b_mlp = np.asarray(b_mlp, np.float32)
    preds = np.empty((T, B_TOTAL, POSE), np.float32)
    for c in range(NCORES):
        arr = np.asarray(per_core_out[c], np.float32)
        a = (
            arr.reshape(T, NSC, 2, 2, POSE, C)
            .transpose(0, 1, 2, 3, 5, 4)
            .reshape(T, BS, POSE)
        )
        preds[:, c * BS : (c + 1) * BS] = a + b_mlp
    return preds


def kernel(obs_s, latent, W_ih, W_hh, b_ih, b_hh, W_fc, b_fc, W_mlp, b_mlp, pred_len):
    assert int(pred_len) == T, f"kernel hardcodes pred_len={T}, got {pred_len}"
    in_maps = prep_inputs(
        obs_s, latent, W_ih, W_hh, b_ih, b_hh, W_fc, b_fc, W_mlp, b_mlp
    )
    nc = build_nc()
    res = run_bass_kernel_spmd(nc, in_maps, core_ids=list(range(NCORES)))
    return assemble_output([res.results[c]["out"] for c in range(NCORES)], b_mlp)
